# revision 9
# baseline (speedup 1.0000x reference)
"""Trainium2 Bass kernel for nn_Attention_38276748542551 — v3.

Llama-style GQA attention block (DIM=4096, 32 q-heads, 8 kv-heads, hd=128,
b=2, s=2048, start_pos=0), tensor-parallel over heads across 8 NeuronCores:
each core owns 4 q-heads / 1 kv-head and computes a full [b*s, 4096] partial
of the wo output; the all-reduce is done on the host after gathering the 8
partials (fp16).

v3 changes vs v2 (796us):
  - causal diagonal-band trimming: score matmul + mask add + exp run only on
    the valid column range [v*128, 512) of each diagonal tile; the masked-off
    range of the exp tile is zeroed by a GpSimd memset so the (full-width) PV
    matmul and row-sum tree stay correct. PV keeps full 512 columns because
    PSUM start/stop accumulation groups cannot express overlapping column
    ranges.
  - rope pair-swap: the [128,512] permutation matmuls (40 on PE) are replaced
    by two SBUF->SBUF DMAs per tensor (64-partition rotation); PE freed.
  - whole rope pipeline in bf16 (PSUM copies, swap DMAs, cos/sin tables,
    muls/adds): half the DVE/ACT time and bytes of the fp32 version.
  - per-g epilogue reordered: V-projection PSUM is drained first (V emitted
    first in each k-tile group) so the PE transposes right after the last
    proj matmul; rope muls/adds trail on DVE and overlap the next block's
    projections. Phase-1 -> phase-2 transition no longer waits on rope.
  - startup DMA ladder: weights/x arrive in small k-tile chunks so the first
    matmuls start as early as possible.

Dataflow per core (feature-major, moving dim = 512 tokens):
  phase 1 (per 512-token block g): Q/K/V projections (weights stationary,
  x^T moving) -> rope in feature-major form -> Q^T resident in SBUF,
  K^T -> kt_sb, V PE-transposed to token-major -> v_sb (all bf16).
  phase 2 (per batch, per 512-query block, per head): scores TRANSPOSED
  [sk, sq] per 128-sk-tile (trimmed to valid cols) -> causal mask add
  (triangle, DVE) -> exp (ACT, bf16 out) -> PV accumulation on PE + pairwise
  row-sum tree on DVE -> ones-matmul reduce+broadcast -> reciprocal ->
  attn^T bf16. wo chunks of the previous block interleave into the score
  stream to keep the PE saturated.
  phase 3 (per 128-token tile): wo projection -> fp16 partial out.
"""
import sys
import numpy as np
import ml_dtypes

sys.path.insert(0, "/opt/trn_rl_repo")

import concourse.bass as bass  # noqa: E402
import concourse.tile as tile  # noqa: E402
from concourse import bacc, mybir  # noqa: E402
from concourse import bass_utils  # noqa: E402

F32 = mybir.dt.float32
BF16 = mybir.dt.bfloat16
FP16 = mybir.dt.float16
AF = mybir.ActivationFunctionType
BF = ml_dtypes.bfloat16

DIM = 4096
NK = DIM // 128          # contraction k-tiles (32)
NKQ = 4                  # k quarters
KPQ = NK // NKQ          # k-tiles per quarter (8)
HD = 128                 # head dim
NH_LOC = 4               # q heads per core
QDIM = NH_LOC * HD       # 512
KVDIM = 2 * HD           # K and V projected together, 256
N_CORES = 8
SOFTMAX_SCALE = 1.0 / np.sqrt(HD)


def build_nc(B=2, S=2048):
    """Build the per-core Bass program (identical across cores; data differs)."""
    NT = B * S // 128            # 128-token tiles total
    TPB = S // 128               # 128-token tiles per batch
    NQB = S // 512               # 512-token blocks per batch
    NTG = B * NQB                # 512-token blocks total

    nc = bacc.Bacc("TRN2", target_bir_lowering=False, debug=False,
                   enable_asserts=False, num_devices=N_CORES)

    x_t = nc.dram_tensor("x_t", [NTG, NKQ, 128, KPQ, 512], BF16, kind="ExternalInput").ap()
    wq_t = nc.dram_tensor("wq_t", [128, NK, QDIM], BF16, kind="ExternalInput").ap()
    wkv_t = nc.dram_tensor("wkv_t", [128, NK, KVDIM], BF16, kind="ExternalInput").ap()
    wo_t = nc.dram_tensor("wo_t", [QDIM, DIM], BF16, kind="ExternalInput").ap()
    cct_d = nc.dram_tensor("cct", [128, S], BF16, kind="ExternalInput").ap()
    sst_d = nc.dram_tensor("sst", [128, S], BF16, kind="ExternalInput").ap()
    ident_d = nc.dram_tensor("ident", [128, 128], BF16, kind="ExternalInput").ap()
    ones_d = nc.dram_tensor("ones", [128, 128], BF16, kind="ExternalInput").ap()
    mask_d = nc.dram_tensor("mask", [128, 128], F32, kind="ExternalInput").ap()
    out_d = nc.dram_tensor("out", [B * S, DIM], FP16, kind="ExternalOutput").ap()

    with tile.TileContext(nc) as tc:
        with tc.tile_pool(name="singles", bufs=1) as singles, \
             tc.tile_pool(name="qts", bufs=1) as qts, \
             tc.tile_pool(name="p1f", bufs=1) as p1f, \
             tc.tile_pool(name="p1r", bufs=2) as p1r, \
             tc.tile_pool(name="p1t", bufs=2) as p1t:
            ident = singles.tile([128, 128], BF16)
            ones_sb = singles.tile([128, 128], BF16)
            kt_sb = singles.tile([128, NT, 128], BF16)   # K^T: [hd, tile, tok]
            v_sb = singles.tile([128, NT, 128], BF16)    # V: [tok, tile, hd]
            qt_sb = qts.tile([128, NTG, NH_LOC, 512], BF16)  # Q^T resident
            # phase-2 weights, prefetched during phase 1 (DMA queue has slack)
            mask_sb = singles.tile([128, 128], F32)
            wo_sb = singles.tile([128, NH_LOC, DIM], BF16)

            # rope muls/adds of the LAST block are deferred into phase 2 (their
            # outputs are only read by the final (b=1,qb=3) score block): at the
            # phase boundary the DVE queue otherwise serializes rope -> tree
            # adds -> exp -> PE. Emitted via deferred_work, one group per
            # h-section starting at block 2.
            deferred_work = []

            # ---------------- phase 1: projections + rope (feature-major) ----------------
            with tc.tile_pool(name="p1w", bufs=1) as p1w, \
                 tc.tile_pool(name="p1", bufs=5) as p1, \
                 tc.tile_pool(name="ps_acc", bufs=6, space="PSUM") as ps_accp, \
                 tc.tile_pool(name="ps_misc", bufs=2, space="PSUM") as ps_miscp:

                def load_xs(g, kq):
                    t_ = p1.tile([128, KPQ, 512], BF16, tag="xs")
                    nc.sync.dma_start(out=t_, in_=x_t[g, kq])
                    return t_

                # startup ladder: per-k-tile chunks so the first matmuls can
                # start while the bulk still streams
                wq_sb = p1w.tile([128, NK, QDIM], BF16)
                wkv_sb = p1w.tile([128, NK, KVDIM], BF16)
                xs00 = p1.tile([128, KPQ, 512], BF16, tag="xs")
                xs_pre = [xs00]

                def load_wq(k0, k1):
                    nc.sync.dma_start(
                        out=wq_sb[:, k0:k1, :], in_=wq_t[:, k0:k1, :])

                def load_wkv(k0, k1):
                    nc.sync.dma_start(
                        out=wkv_sb[:, k0:k1, :], in_=wkv_t[:, k0:k1, :])

                ladder = [0, 1, 2, 3, 4, 6, 8]
                for i in range(len(ladder) - 1):
                    k0, k1 = ladder[i], ladder[i + 1]
                    nc.sync.dma_start(out=xs00[:, k0:k1, :], in_=x_t[0, 0, :, k0:k1, :])
                    load_wq(k0, k1)
                    load_wkv(k0, k1)
                # xs quarter 1 in halves so its first k-tiles land early
                xsq1 = p1.tile([128, KPQ, 512], BF16, tag="xs")
                nc.sync.dma_start(out=xsq1[:, 0:4, :], in_=x_t[0, 1, :, 0:4, :])
                load_wq(8, 16)
                nc.sync.dma_start(out=xsq1[:, 4:8, :], in_=x_t[0, 1, :, 4:8, :])
                xs_pre.append(xsq1)
                load_wkv(8, 16)
                xs_pre.append(load_xs(0, 2))
                load_wq(16, 24)
                load_wkv(16, 24)
                xs_pre.append(load_xs(0, 3))
                load_wq(24, 32)
                load_wkv(24, 32)
                nc.sync.dma_start(out=ident, in_=ident_d)
                nc.sync.dma_start(out=ones_sb, in_=ones_d)
                nc.sync.dma_start(out=mask_sb, in_=mask_d)

                nload = 4   # next (g*NKQ+kq) index to load; keep 3 in flight
                for g in range(NTG):
                    if g == 1:
                        # wo prefetch: 4MB, first needed at ~360us -- keep it
                        # out of the startup DMA contention window
                        for kk in range(NH_LOC):
                            nc.sync.dma_start(
                                out=wo_sb[:, kk, :],
                                in_=wo_t[kk * 128:(kk + 1) * 128, :])
                    # per-block rope table slices [128, 512]
                    pos = (g % NQB) * 512
                    cct = p1t.tile([128, 512], BF16, tag="cct")
                    nc.sync.dma_start(out=cct, in_=cct_d[:, pos:pos + 512])
                    sst = p1t.tile([128, 512], BF16, tag="sst")
                    nc.sync.dma_start(out=sst, in_=sst_d[:, pos:pos + 512])

                    acc = [ps_accp.tile([128, 512], F32, tag="acc", name=f"acc{g}_{j}") for j in range(6)]
                    for kq in range(NKQ):
                        xs = xs_pre.pop(0)
                        if nload < NTG * NKQ:
                            xs_pre.append(load_xs(nload // NKQ, nload % NKQ))
                            nload += 1
                        for k in range(KPQ):
                            kt = kq * KPQ + k
                            st = (kt == 0)
                            sp = (kt == NK - 1)
                            # V first so its PSUM stops earliest (drain order)
                            nc.tensor.matmul(acc[5], wkv_sb[:, kt, 128:256],
                                             xs[:, k, :], start=st, stop=sp)
                            for h in range(NH_LOC):
                                nc.tensor.matmul(acc[h], wq_sb[:, kt, h * 128:(h + 1) * 128],
                                                 xs[:, k, :], start=st, stop=sp)
                            nc.tensor.matmul(acc[4], wkv_sb[:, kt, 0:128],
                                             xs[:, k, :], start=st, stop=sp)

                    # epilogue: drain PSUM to bf16, swap via SBUF->SBUF DMA,
                    # V transpose on PE right after the projections; rope
                    # muls/adds trail on DVE under the next block's matmuls
                    vf = p1f.tile([128, 512], BF16, tag="vf")
                    nc.vector.tensor_copy(vf, acc[5])
                    fs, sws = [], []
                    for j in range(5):   # 0..3 = q heads, 4 = K
                        f = p1f.tile([128, 512], BF16, tag=f"f{j}")
                        if j % 2 == 0:
                            nc.scalar.copy(f, acc[j])
                        else:
                            nc.vector.tensor_copy(f, acc[j])
                        sw = p1f.tile([128, 512], BF16, tag=f"sw{j}")
                        nc.sync.dma_start(out=sw[0:64, :], in_=f[64:128, :])
                        nc.sync.dma_start(out=sw[64:128, :], in_=f[0:64, :])
                        fs.append(f)
                        sws.append(sw)
                    for r in range(4):
                        ps_vt = ps_miscp.tile([128, 128], BF16, tag="misc")
                        nc.tensor.transpose(ps_vt, vf[:, r * 128:(r + 1) * 128], ident)
                        nc.scalar.copy(v_sb[:, 4 * g + r, :], ps_vt)
                    def rope_group(g, j, f, sw, cct, sst):
                        t1 = p1r.tile([128, 512], BF16, tag="t1")
                        nc.vector.tensor_mul(t1, f, cct)
                        t2 = p1r.tile([128, 512], BF16, tag="t2")
                        nc.vector.tensor_mul(t2, sw, sst)
                        if j < NH_LOC:
                            nc.vector.tensor_add(qt_sb[:, g, j, :], t1, t2)
                        else:
                            nc.vector.tensor_add(
                                kt_sb[:, 4 * g:4 * g + 4, :].rearrange("p a t -> p (a t)"),
                                t1, t2)

                    for j in range(5):
                        if g == NTG - 1:
                            deferred_work.append(
                                lambda g=g, j=j, f=fs[j], sw=sws[j], cct=cct,
                                       sst=sst: rope_group(g, j, f, sw, cct, sst))
                        else:
                            rope_group(g, j, fs[j], sws[j], cct, sst)

            # ------------- phase 2/3: attention (transposed scores) + wo -------------
            with tc.tile_pool(name="p2", bufs=2) as p2, \
                 tc.tile_pool(name="p2e", bufs=6) as p2e, \
                 tc.tile_pool(name="p2tr", bufs=12) as p2tr, \
                 tc.tile_pool(name="p2l", bufs=2) as p2l, \
                 tc.tile_pool(name="ps_s", bufs=4, space="PSUM") as ps_sp, \
                 tc.tile_pool(name="ps_o", bufs=2, space="PSUM") as ps_op, \
                 tc.tile_pool(name="ps_w", bufs=2, space="PSUM") as ps_wp:

                # wo is emitted as chunk closures per block, spread evenly
                # through the NEXT block's score stream: the score sections
                # alone are ACT-bound (exp > PE per tile), and monolithic wo
                # sections were PE-bound while ACT idled. In the final drain
                # (no score stream) chunks alternate between the ps_w and
                # ps_s pools so four banks pipeline.
                def build_wo_chunks(b, qb, attn_t, drain=False):
                    chunks = []
                    for r in range(4):
                        tt = b * TPB + qb * 4 + r
                        holder = {}
                        for n in range(DIM // 512):
                            for half in range(2):
                                def ch(r=r, n=n, tt=tt, half=half,
                                       holder=holder, attn_t=attn_t):
                                    if half == 0:
                                        if n == 0:
                                            holder["o"] = p2.tile(
                                                [128, DIM], FP16, tag="o_sb",
                                                name=f"o_sb_{tt}")
                                        pool = ps_sp if drain and n % 2 else ps_wp
                                        tg = "ps_s" if drain and n % 2 else "ps_w"
                                        holder["w"] = pool.tile(
                                            [128, 512], F32, tag=tg,
                                            name=f"ps_w_{tt}_{n}")
                                    o_sb = holder["o"]
                                    ps_w = holder["w"]
                                    for kk in (0, 1) if half == 0 else (2, 3):
                                        nc.tensor.matmul(ps_w, attn_t[:, kk, r, :],
                                                         wo_sb[:, kk, n * 512:(n + 1) * 512],
                                                         start=(kk == 0), stop=(kk == NH_LOC - 1))
                                    if half == 1:
                                        if n % 2 == 0:   # split PSUM->fp16 casts DVE/ACT
                                            nc.vector.tensor_copy(
                                                o_sb[:, n * 512:(n + 1) * 512], ps_w)
                                        else:
                                            nc.scalar.copy(
                                                o_sb[:, n * 512:(n + 1) * 512], ps_w)
                                        if n == DIM // 512 - 1:
                                            nc.sync.dma_start(
                                                out=out_d[tt * 128:(tt + 1) * 128, :], in_=o_sb)
                                chunks.append(ch)
                    return chunks

                SKIP = 3   # slots at block start with no wo chunks (attn_t lag)
                pending_chunks = []
                for b in range(B):
                    for qb in range(NQB):
                        g = b * NQB + qb
                        nt = 4 * (qb + 1)            # sk tiles for this block
                        attn_t = p2.tile([128, NH_LOC, 4, 128], BF16, tag="attn_t")
                        total_slots = NH_LOC * nt
                        n_chunks = len(pending_chunks)
                        emitted = 0
                        slot = 0
                        for h in range(NH_LOC):
                            ps_o = ps_op.tile([128, 512], F32, tag="ps_o")
                            # software pipeline: score(t) issues on PE before
                            # PV(t-1) so exp(t-1) hides under score(t)
                            ets = []
                            partials = []
                            for t in range(nt):
                                v = t - 4 * qb
                                c0 = v * 128 if v > 0 else 0
                                ps_s = ps_sp.tile([128, 512], F32, tag="ps_s")
                                nc.tensor.matmul(ps_s[:, c0:], kt_sb[:, b * TPB + t, :],
                                                 qt_sb[:, g, h, c0:],
                                                 start=True, stop=True)
                                if v >= 0:   # diagonal band: causal triangle
                                    nc.vector.tensor_add(ps_s[:, c0:c0 + 128],
                                                         ps_s[:, c0:c0 + 128], mask_sb)
                                et = p2e.tile([128, 512], BF16, tag="et")
                                if c0 > 0:
                                    nc.gpsimd.memset(et[:, 0:c0], 0)
                                nc.scalar.activation(et[:, c0:], ps_s[:, c0:],
                                                     AF.Exp, scale=SOFTMAX_SCALE)
                                ets.append(et)
                                if t >= 1:   # deferred PV of the previous tile
                                    nc.tensor.matmul(ps_o, v_sb[:, b * TPB + t - 1, :],
                                                     ets[t - 1],
                                                     start=(t - 1 == 0), stop=False)
                                if t % 2 == 1:   # exp row-sum: pairwise tree,
                                    # alternating DVE/GpSimd to halve DVE load
                                    pt = p2tr.tile([128, 512], BF16, tag="tree")
                                    eng = nc.gpsimd if (t // 2) % 2 == 0 else nc.vector
                                    eng.tensor_add(pt, ets[t - 1], ets[t])
                                    partials.append(pt)
                                # spread the previous block's wo chunks evenly
                                slot += 1
                                if n_chunks and slot > SKIP:
                                    want = n_chunks * (slot - SKIP) // (total_slots - SKIP)
                                    while emitted < want:
                                        pending_chunks[emitted]()
                                        emitted += 1
                            nc.tensor.matmul(ps_o, v_sb[:, b * TPB + nt - 1, :],
                                             ets[nt - 1], start=False, stop=True)
                            lvl = partials
                            while len(lvl) > 1:
                                nxt = []
                                for i in range(0, len(lvl) - 1, 2):
                                    o = p2tr.tile([128, 512], BF16, tag="tree")
                                    nc.vector.tensor_add(o, lvl[i], lvl[i + 1])
                                    nxt.append(o)
                                if len(lvl) % 2:
                                    nxt.append(lvl[-1])
                                lvl = nxt
                            # reduce over sk partitions AND broadcast to 128 rows
                            ps_b = ps_wp.tile([128, 512], F32, tag="ps_w", name=f"ps_b{g}_{h}")
                            nc.tensor.matmul(ps_b, ones_sb, lvl[0], start=True, stop=True)
                            rb = p2l.tile([128, 512], F32, tag="rb")
                            nc.vector.reciprocal_approx_fast(out=rb, in_=ps_b)
                            nc.vector.tensor_mul(
                                attn_t[:, h].rearrange("p r t -> p (r t)"), ps_o, rb)
                            # deferred last-block rope: one group per h-section
                            # once the pipeline is warm (block >= 2); all groups
                            # land before the (1,3) block that reads them
                            if g >= 2 and deferred_work:
                                deferred_work.pop(0)()
                        while emitted < n_chunks:   # safety drain
                            pending_chunks[emitted]()
                            emitted += 1
                        pending_chunks = build_wo_chunks(
                            b, qb, attn_t, drain=(b == B - 1 and qb == NQB - 1))
                for ch in pending_chunks:   # last block's wo
                    ch()

    nc.compile()
    return nc


def host_prepare(x, wq, wk, wv, wo, freqs_cos, freqs_sin, B, S):
    """Build per-core in_maps. Weights nn.Linear-style [out, in]."""
    NQB = S // 512
    NTG = B * NQB
    n_heads = wq.shape[0] // HD
    n_kv = wk.shape[0] // HD
    hpc = n_heads // N_CORES       # q heads per core (4)
    kpc = n_kv // N_CORES          # kv heads per core (1)

    # deinterleave rope pairs: feature order (2i) first then (2i+1), per head
    de = np.concatenate([np.arange(0, HD, 2), np.arange(1, HD, 2)])

    xf = np.ascontiguousarray(x.reshape(B * S, DIM))
    # x^T tiled: [g, kq, p, k, t] (partition-major so the DMA is contiguous)
    x_t = np.ascontiguousarray(
        xf.T.reshape(NKQ, KPQ, 128, NTG, 512).transpose(3, 0, 2, 1, 4)).astype(BF)

    cos = np.repeat(freqs_cos, 2, axis=1)   # [S, 128] interleaved dup
    sin = np.repeat(freqs_sin, 2, axis=1)
    cc = cos[:, de]                                             # deinterleaved
    ss = sin.copy()
    ss[:, 0::2] *= -1.0                                         # [-sin, +sin]
    ss = ss[:, de]
    cct = np.ascontiguousarray(cc.T)                            # [128, S]
    sst = np.ascontiguousarray(ss.T)

    ident = np.eye(128, dtype=np.float32)
    ones = np.ones((128, 128), dtype=np.float32)
    # transposed-orientation causal triangle: scores^T [sk r, sq j] within a
    # 128x128 diagonal sub-block -- identical for every diagonal tile
    r_idx = np.arange(128)[:, None]
    j_idx = np.arange(128)[None, :]
    mask = np.where(r_idx <= j_idx, 0.0, -1e30).astype(np.float32)

    in_maps = []
    for cidx in range(N_CORES):
        qs = slice(cidx * hpc * HD, (cidx + 1) * hpc * HD)
        ks = slice(cidx * kpc * HD, (cidx + 1) * kpc * HD)
        wq_c = wq[qs].reshape(hpc, HD, DIM)[:, de, :].reshape(hpc * HD, DIM)
        wk_c = wk[ks].reshape(kpc, HD, DIM)[:, de, :].reshape(kpc * HD, DIM)
        wv_c = wv[ks]
        wkv_c = np.concatenate([wk_c, wv_c], axis=0)
        wo_c = wo[:, qs]
        in_maps.append({
            "x_t": x_t,
            "wq_t": np.ascontiguousarray(
                wq_c.T.reshape(NK, 128, hpc * HD).transpose(1, 0, 2)).astype(BF),
            "wkv_t": np.ascontiguousarray(
                wkv_c.T.reshape(NK, 128, KVDIM).transpose(1, 0, 2)).astype(BF),
            "wo_t": np.ascontiguousarray(wo_c.T).astype(BF),
            "cct": cct.astype(BF),
            "sst": sst.astype(BF),
            "ident": ident.astype(BF),
            "ones": ones.astype(BF),
            "mask": mask,
        })
    return in_maps


_CACHE = {}


def run(inputs, trace=False, trace_cores=None):
    x = np.asarray(inputs["x"], dtype=np.float32)
    B, S, _ = x.shape
    key = (B, S)
    if key not in _CACHE:
        _CACHE[key] = build_nc(B, S)
    nc = _CACHE[key]
    in_maps = host_prepare(
        x, np.asarray(inputs["wq"], np.float32), np.asarray(inputs["wk"], np.float32),
        np.asarray(inputs["wv"], np.float32), np.asarray(inputs["wo"], np.float32),
        np.asarray(inputs["freqs_cos"], np.float32),
        np.asarray(inputs["freqs_sin"], np.float32), B, S)
    res = bass_utils.run_bass_kernel_spmd(
        nc, in_maps, core_ids=list(range(N_CORES)), trace=trace,
        trace_cores=trace_cores)
    acc = np.zeros((B * S, DIM), dtype=np.float64)
    for r in res.results:
        acc += r["out"].astype(np.float64)
    out = acc.astype(np.float32).reshape(B, S, DIM)
    return out, res


def kernel(**inputs) -> np.ndarray:
    assert int(inputs.get("start_pos", 0)) == 0
    out, _ = run(inputs, trace=False)
    return out


# revision 10
# speedup vs baseline: 1.0269x; 1.0269x over previous
"""Trainium2 Bass kernel for nn_Attention_38276748542551 — v3.

Llama-style GQA attention block (DIM=4096, 32 q-heads, 8 kv-heads, hd=128,
b=2, s=2048, start_pos=0), tensor-parallel over heads across 8 NeuronCores:
each core owns 4 q-heads / 1 kv-head and computes a full [b*s, 4096] partial
of the wo output; the all-reduce is done on the host after gathering the 8
partials (fp16).

v3 changes vs v2 (796us):
  - causal diagonal-band trimming: score matmul + mask add + exp run only on
    the valid column range [v*128, 512) of each diagonal tile; the masked-off
    range of the exp tile is zeroed by a GpSimd memset so the (full-width) PV
    matmul and row-sum tree stay correct. PV keeps full 512 columns because
    PSUM start/stop accumulation groups cannot express overlapping column
    ranges.
  - rope pair-swap: the [128,512] permutation matmuls (40 on PE) are replaced
    by two SBUF->SBUF DMAs per tensor (64-partition rotation); PE freed.
  - whole rope pipeline in bf16 (PSUM copies, swap DMAs, cos/sin tables,
    muls/adds): half the DVE/ACT time and bytes of the fp32 version.
  - per-g epilogue reordered: V-projection PSUM is drained first (V emitted
    first in each k-tile group) so the PE transposes right after the last
    proj matmul; rope muls/adds trail on DVE and overlap the next block's
    projections. Phase-1 -> phase-2 transition no longer waits on rope.
  - startup DMA ladder: weights/x arrive in small k-tile chunks so the first
    matmuls start as early as possible.

Dataflow per core (feature-major, moving dim = 512 tokens):
  phase 1 (per 512-token block g): Q/K/V projections (weights stationary,
  x^T moving) -> rope in feature-major form -> Q^T resident in SBUF,
  K^T -> kt_sb, V PE-transposed to token-major -> v_sb (all bf16).
  phase 2 (per batch, per 512-query block, per head): scores TRANSPOSED
  [sk, sq] per 128-sk-tile (trimmed to valid cols) -> causal mask add
  (triangle, DVE) -> exp (ACT, bf16 out) -> PV accumulation on PE + pairwise
  row-sum tree on DVE -> ones-matmul reduce+broadcast -> reciprocal ->
  attn^T bf16. wo chunks of the previous block interleave into the score
  stream to keep the PE saturated.
  phase 3 (per 128-token tile): wo projection -> fp16 partial out.
"""
import sys
import numpy as np
import ml_dtypes

sys.path.insert(0, "/opt/trn_rl_repo")

import concourse.bass as bass  # noqa: E402
import concourse.tile as tile  # noqa: E402
from concourse import bacc, mybir  # noqa: E402
from concourse import bass_utils  # noqa: E402

F32 = mybir.dt.float32
BF16 = mybir.dt.bfloat16
FP16 = mybir.dt.float16
AF = mybir.ActivationFunctionType
BF = ml_dtypes.bfloat16

DIM = 4096
NK = DIM // 128          # contraction k-tiles (32)
NKQ = 4                  # k quarters
KPQ = NK // NKQ          # k-tiles per quarter (8)
HD = 128                 # head dim
NH_LOC = 4               # q heads per core
QDIM = NH_LOC * HD       # 512
KVDIM = 2 * HD           # K and V projected together, 256
N_CORES = 8
SOFTMAX_SCALE = 1.0 / np.sqrt(HD)


def build_nc(B=2, S=2048):
    """Build the per-core Bass program (identical across cores; data differs)."""
    NT = B * S // 128            # 128-token tiles total
    TPB = S // 128               # 128-token tiles per batch
    NQB = S // 512               # 512-token blocks per batch
    NTG = B * NQB                # 512-token blocks total

    nc = bacc.Bacc("TRN2", target_bir_lowering=False, debug=False,
                   enable_asserts=False, num_devices=N_CORES)

    x_t = nc.dram_tensor("x_t", [NTG, NKQ, 128, KPQ, 512], BF16, kind="ExternalInput").ap()
    wq_t = nc.dram_tensor("wq_t", [128, NK, QDIM], BF16, kind="ExternalInput").ap()
    wkv_t = nc.dram_tensor("wkv_t", [128, NK, KVDIM], BF16, kind="ExternalInput").ap()
    wo_t = nc.dram_tensor("wo_t", [QDIM, DIM], BF16, kind="ExternalInput").ap()
    cct_d = nc.dram_tensor("cct", [128, S], BF16, kind="ExternalInput").ap()
    sst_d = nc.dram_tensor("sst", [128, S], BF16, kind="ExternalInput").ap()
    ident_d = nc.dram_tensor("ident", [128, 128], BF16, kind="ExternalInput").ap()
    ones_d = nc.dram_tensor("ones", [128, 128], BF16, kind="ExternalInput").ap()
    mask_d = nc.dram_tensor("mask", [128, 128], F32, kind="ExternalInput").ap()
    out_d = nc.dram_tensor("out", [B * S, DIM], FP16, kind="ExternalOutput").ap()

    with tile.TileContext(nc) as tc:
        with tc.tile_pool(name="singles", bufs=1) as singles, \
             tc.tile_pool(name="qts", bufs=1) as qts, \
             tc.tile_pool(name="p1f", bufs=1) as p1f, \
             tc.tile_pool(name="p1r", bufs=2) as p1r, \
             tc.tile_pool(name="p1t", bufs=2) as p1t:
            ident = singles.tile([128, 128], BF16)
            ones_sb = singles.tile([128, 128], BF16)
            kt_sb = singles.tile([128, NT, 128], BF16)   # K^T: [hd, tile, tok]
            v_sb = singles.tile([128, NT, 128], BF16)    # V: [tok, tile, hd]
            qt_sb = qts.tile([128, NTG, NH_LOC, 512], BF16)  # Q^T resident
            # phase-2 weights, prefetched during phase 1 (DMA queue has slack)
            mask_sb = singles.tile([128, 128], F32)
            wo_sb = singles.tile([128, NH_LOC, DIM], BF16)

            # rope muls/adds of the LAST block are deferred into phase 2 (their
            # outputs are only read by the final (b=1,qb=3) score block): at the
            # phase boundary the DVE queue otherwise serializes rope -> tree
            # adds -> exp -> PE. Emitted via deferred_work, one group per
            # h-section starting at block 2.
            deferred_work = []

            # ---------------- phase 1: projections + rope (feature-major) ----------------
            with tc.tile_pool(name="p1w", bufs=1) as p1w, \
                 tc.tile_pool(name="p1", bufs=5) as p1, \
                 tc.tile_pool(name="ps_acc", bufs=6, space="PSUM") as ps_accp, \
                 tc.tile_pool(name="ps_misc", bufs=2, space="PSUM") as ps_miscp:

                def load_xs(g, kq):
                    t_ = p1.tile([128, KPQ, 512], BF16, tag="xs")
                    nc.sync.dma_start(out=t_, in_=x_t[g, kq])
                    return t_

                # startup ladder: per-k-tile chunks so the first matmuls can
                # start while the bulk still streams
                wq_sb = p1w.tile([128, NK, QDIM], BF16)
                wkv_sb = p1w.tile([128, NK, KVDIM], BF16)
                xs00 = p1.tile([128, KPQ, 512], BF16, tag="xs")
                xs_pre = [xs00]

                def load_wq(k0, k1):
                    nc.sync.dma_start(
                        out=wq_sb[:, k0:k1, :], in_=wq_t[:, k0:k1, :])

                def load_wkv(k0, k1):
                    nc.sync.dma_start(
                        out=wkv_sb[:, k0:k1, :], in_=wkv_t[:, k0:k1, :])

                ladder = [0, 1, 2, 3, 4, 6, 8]
                for i in range(len(ladder) - 1):
                    k0, k1 = ladder[i], ladder[i + 1]
                    nc.sync.dma_start(out=xs00[:, k0:k1, :], in_=x_t[0, 0, :, k0:k1, :])
                    load_wq(k0, k1)
                    load_wkv(k0, k1)
                # xs quarter 1 in halves so its first k-tiles land early
                xsq1 = p1.tile([128, KPQ, 512], BF16, tag="xs")
                nc.sync.dma_start(out=xsq1[:, 0:4, :], in_=x_t[0, 1, :, 0:4, :])
                load_wq(8, 16)
                nc.sync.dma_start(out=xsq1[:, 4:8, :], in_=x_t[0, 1, :, 4:8, :])
                xs_pre.append(xsq1)
                load_wkv(8, 16)
                xs_pre.append(load_xs(0, 2))
                load_wq(16, 24)
                load_wkv(16, 24)
                xs_pre.append(load_xs(0, 3))
                load_wq(24, 32)
                load_wkv(24, 32)
                nc.sync.dma_start(out=ident, in_=ident_d)
                nc.sync.dma_start(out=ones_sb, in_=ones_d)
                nc.sync.dma_start(out=mask_sb, in_=mask_d)

                nload = 4   # next (g*NKQ+kq) index to load; keep 3 in flight
                for g in range(NTG):
                    if g == 1:
                        # wo prefetch: 4MB, first needed at ~360us -- keep it
                        # out of the startup DMA contention window
                        for kk in range(NH_LOC):
                            nc.sync.dma_start(
                                out=wo_sb[:, kk, :],
                                in_=wo_t[kk * 128:(kk + 1) * 128, :])
                    # per-block rope table slices [128, 512]
                    pos = (g % NQB) * 512
                    cct = p1t.tile([128, 512], BF16, tag="cct")
                    nc.sync.dma_start(out=cct, in_=cct_d[:, pos:pos + 512])
                    sst = p1t.tile([128, 512], BF16, tag="sst")
                    nc.sync.dma_start(out=sst, in_=sst_d[:, pos:pos + 512])

                    acc = [ps_accp.tile([128, 512], F32, tag="acc", name=f"acc{g}_{j}") for j in range(6)]
                    for kq in range(NKQ):
                        xs = xs_pre.pop(0)
                        if nload < NTG * NKQ:
                            xs_pre.append(load_xs(nload // NKQ, nload % NKQ))
                            nload += 1
                        for k in range(KPQ):
                            kt = kq * KPQ + k
                            st = (kt == 0)
                            sp = (kt == NK - 1)
                            # V first so its PSUM stops earliest (drain order)
                            nc.tensor.matmul(acc[5], wkv_sb[:, kt, 128:256],
                                             xs[:, k, :], start=st, stop=sp)
                            for h in range(NH_LOC):
                                nc.tensor.matmul(acc[h], wq_sb[:, kt, h * 128:(h + 1) * 128],
                                                 xs[:, k, :], start=st, stop=sp)
                            nc.tensor.matmul(acc[4], wkv_sb[:, kt, 0:128],
                                             xs[:, k, :], start=st, stop=sp)

                    # epilogue: drain PSUM to bf16, swap via SBUF->SBUF DMA,
                    # V transpose on PE right after the projections; rope
                    # muls/adds trail on DVE under the next block's matmuls
                    vf = p1f.tile([128, 512], BF16, tag="vf")
                    nc.vector.tensor_copy(vf, acc[5])
                    fs, sws = [], []
                    for j in range(5):   # 0..3 = q heads, 4 = K
                        f = p1f.tile([128, 512], BF16, tag=f"f{j}")
                        if j % 2 == 0:
                            nc.scalar.copy(f, acc[j])
                        else:
                            nc.vector.tensor_copy(f, acc[j])
                        sw = p1f.tile([128, 512], BF16, tag=f"sw{j}")
                        nc.sync.dma_start(out=sw[0:64, :], in_=f[64:128, :])
                        nc.sync.dma_start(out=sw[64:128, :], in_=f[0:64, :])
                        fs.append(f)
                        sws.append(sw)
                    for r in range(4):
                        ps_vt = ps_miscp.tile([128, 128], BF16, tag="misc")
                        nc.tensor.transpose(ps_vt, vf[:, r * 128:(r + 1) * 128], ident)
                        nc.scalar.copy(v_sb[:, 4 * g + r, :], ps_vt)
                    def rope_group(g, j, f, sw, cct, sst):
                        t1 = p1r.tile([128, 512], BF16, tag="t1")
                        nc.vector.tensor_mul(t1, f, cct)
                        t2 = p1r.tile([128, 512], BF16, tag="t2")
                        nc.vector.tensor_mul(t2, sw, sst)
                        if j < NH_LOC:
                            nc.vector.tensor_add(qt_sb[:, g, j, :], t1, t2)
                        else:
                            nc.vector.tensor_add(
                                kt_sb[:, 4 * g:4 * g + 4, :].rearrange("p a t -> p (a t)"),
                                t1, t2)

                    for j in range(5):
                        if g == NTG - 1:
                            deferred_work.append(
                                lambda g=g, j=j, f=fs[j], sw=sws[j], cct=cct,
                                       sst=sst: rope_group(g, j, f, sw, cct, sst))
                        else:
                            rope_group(g, j, fs[j], sws[j], cct, sst)

            # ------------- phase 2/3: attention (transposed scores) + wo -------------
            with tc.tile_pool(name="p2", bufs=2) as p2, \
                 tc.tile_pool(name="p2e", bufs=6) as p2e, \
                 tc.tile_pool(name="p2tr", bufs=12) as p2tr, \
                 tc.tile_pool(name="p2l", bufs=2) as p2l, \
                 tc.tile_pool(name="ps_s", bufs=4, space="PSUM") as ps_sp, \
                 tc.tile_pool(name="ps_o", bufs=2, space="PSUM") as ps_op, \
                 tc.tile_pool(name="ps_w", bufs=2, space="PSUM") as ps_wp:

                # wo is emitted as chunk closures per block, spread evenly
                # through the NEXT block's score stream: the score sections
                # alone are ACT-bound (exp > PE per tile), and monolithic wo
                # sections were PE-bound while ACT idled. In the final drain
                # (no score stream) chunks alternate between the ps_w and
                # ps_s pools so four banks pipeline.
                def build_wo_chunks(b, qb, attn_t, drain=False):
                    chunks = []
                    for r in range(4):
                        tt = b * TPB + qb * 4 + r
                        holder = {}
                        for n in range(DIM // 512):
                            for half in range(2):
                                def ch(r=r, n=n, tt=tt, half=half,
                                       holder=holder, attn_t=attn_t):
                                    if half == 0:
                                        if n == 0:
                                            holder["o"] = p2.tile(
                                                [128, DIM], FP16, tag="o_sb",
                                                name=f"o_sb_{tt}")
                                        pool = ps_sp if drain and n % 2 else ps_wp
                                        tg = "ps_s" if drain and n % 2 else "ps_w"
                                        holder["w"] = pool.tile(
                                            [128, 512], F32, tag=tg,
                                            name=f"ps_w_{tt}_{n}")
                                    o_sb = holder["o"]
                                    ps_w = holder["w"]
                                    for kk in (0, 1) if half == 0 else (2, 3):
                                        nc.tensor.matmul(ps_w, attn_t[:, kk, r, :],
                                                         wo_sb[:, kk, n * 512:(n + 1) * 512],
                                                         start=(kk == 0), stop=(kk == NH_LOC - 1))
                                    if half == 1:
                                        if n % 2 == 0:   # split PSUM->fp16 casts DVE/ACT
                                            nc.vector.tensor_copy(
                                                o_sb[:, n * 512:(n + 1) * 512], ps_w)
                                        else:
                                            nc.scalar.copy(
                                                o_sb[:, n * 512:(n + 1) * 512], ps_w)
                                        if n == DIM // 512 - 1:
                                            nc.sync.dma_start(
                                                out=out_d[tt * 128:(tt + 1) * 128, :], in_=o_sb)
                                chunks.append(ch)
                    return chunks

                SKIP = 3   # slots at block start with no wo chunks (attn_t lag)
                pending_chunks = []
                for b in range(B):
                    for qb in range(NQB):
                        g = b * NQB + qb
                        nt = 4 * (qb + 1)            # sk tiles for this block
                        attn_t = p2.tile([128, NH_LOC, 4, 128], BF16, tag="attn_t")
                        total_slots = NH_LOC * nt
                        n_chunks = len(pending_chunks)
                        emitted = 0
                        slot = 0
                        for h in range(NH_LOC):
                            ps_o = ps_op.tile([128, 512], F32, tag="ps_o")
                            # software pipeline: score(t) issues on PE before
                            # PV(t-1) so exp(t-1) hides under score(t)
                            ets = []
                            partials = []
                            for t in range(nt):
                                v = t - 4 * qb
                                c0 = v * 128 if v > 0 else 0
                                ps_s = ps_sp.tile([128, 512], F32, tag="ps_s")
                                nc.tensor.matmul(ps_s[:, c0:], kt_sb[:, b * TPB + t, :],
                                                 qt_sb[:, g, h, c0:],
                                                 start=True, stop=True)
                                if v >= 0:   # diagonal band: causal triangle
                                    nc.vector.tensor_add(ps_s[:, c0:c0 + 128],
                                                         ps_s[:, c0:c0 + 128], mask_sb)
                                et = p2e.tile([128, 512], BF16, tag="et")
                                if c0 > 0:
                                    nc.gpsimd.memset(et[:, 0:c0], 0)
                                nc.scalar.activation(et[:, c0:], ps_s[:, c0:],
                                                     AF.Exp, scale=SOFTMAX_SCALE)
                                ets.append(et)
                                if t >= 1:   # deferred PV of the previous tile
                                    nc.tensor.matmul(ps_o, v_sb[:, b * TPB + t - 1, :],
                                                     ets[t - 1],
                                                     start=(t - 1 == 0), stop=False)
                                if t % 2 == 1:   # exp row-sum: pairwise tree on
                                    # DVE (GpSimd adds measured 1.2us -- too
                                    # slow for the l-chain)
                                    pt = p2tr.tile([128, 512], BF16, tag="tree")
                                    nc.vector.tensor_add(pt, ets[t - 1], ets[t])
                                    partials.append(pt)
                                # spread the previous block's wo chunks evenly
                                slot += 1
                                if n_chunks and slot > SKIP:
                                    want = n_chunks * (slot - SKIP) // (total_slots - SKIP)
                                    while emitted < want:
                                        pending_chunks[emitted]()
                                        emitted += 1
                            nc.tensor.matmul(ps_o, v_sb[:, b * TPB + nt - 1, :],
                                             ets[nt - 1], start=False, stop=True)
                            lvl = partials
                            while len(lvl) > 1:
                                nxt = []
                                for i in range(0, len(lvl) - 1, 2):
                                    o = p2tr.tile([128, 512], BF16, tag="tree")
                                    nc.vector.tensor_add(o, lvl[i], lvl[i + 1])
                                    nxt.append(o)
                                if len(lvl) % 2:
                                    nxt.append(lvl[-1])
                                lvl = nxt
                            # reduce over sk partitions AND broadcast to 128 rows
                            ps_b = ps_wp.tile([128, 512], F32, tag="ps_w", name=f"ps_b{g}_{h}")
                            nc.tensor.matmul(ps_b, ones_sb, lvl[0], start=True, stop=True)
                            rb = p2l.tile([128, 512], F32, tag="rb")
                            nc.vector.reciprocal_approx_fast(out=rb, in_=ps_b)
                            nc.vector.tensor_mul(
                                attn_t[:, h].rearrange("p r t -> p (r t)"), ps_o, rb)
                            # deferred last-block rope: one group per h-section
                            # once the pipeline is warm (block >= 2); all groups
                            # land before the (1,3) block that reads them
                            if g >= 2 and deferred_work:
                                deferred_work.pop(0)()
                        while emitted < n_chunks:   # safety drain
                            pending_chunks[emitted]()
                            emitted += 1
                        pending_chunks = build_wo_chunks(
                            b, qb, attn_t, drain=(b == B - 1 and qb == NQB - 1))
                for ch in pending_chunks:   # last block's wo
                    ch()

    nc.compile()
    return nc


def host_prepare(x, wq, wk, wv, wo, freqs_cos, freqs_sin, B, S):
    """Build per-core in_maps. Weights nn.Linear-style [out, in]."""
    NQB = S // 512
    NTG = B * NQB
    n_heads = wq.shape[0] // HD
    n_kv = wk.shape[0] // HD
    hpc = n_heads // N_CORES       # q heads per core (4)
    kpc = n_kv // N_CORES          # kv heads per core (1)

    # deinterleave rope pairs: feature order (2i) first then (2i+1), per head
    de = np.concatenate([np.arange(0, HD, 2), np.arange(1, HD, 2)])

    xf = np.ascontiguousarray(x.reshape(B * S, DIM))
    # x^T tiled: [g, kq, p, k, t] (partition-major so the DMA is contiguous)
    x_t = np.ascontiguousarray(
        xf.T.reshape(NKQ, KPQ, 128, NTG, 512).transpose(3, 0, 2, 1, 4)).astype(BF)

    cos = np.repeat(freqs_cos, 2, axis=1)   # [S, 128] interleaved dup
    sin = np.repeat(freqs_sin, 2, axis=1)
    cc = cos[:, de]                                             # deinterleaved
    ss = sin.copy()
    ss[:, 0::2] *= -1.0                                         # [-sin, +sin]
    ss = ss[:, de]
    cct = np.ascontiguousarray(cc.T)                            # [128, S]
    sst = np.ascontiguousarray(ss.T)

    ident = np.eye(128, dtype=np.float32)
    ones = np.ones((128, 128), dtype=np.float32)
    # transposed-orientation causal triangle: scores^T [sk r, sq j] within a
    # 128x128 diagonal sub-block -- identical for every diagonal tile
    r_idx = np.arange(128)[:, None]
    j_idx = np.arange(128)[None, :]
    mask = np.where(r_idx <= j_idx, 0.0, -1e30).astype(np.float32)

    in_maps = []
    for cidx in range(N_CORES):
        qs = slice(cidx * hpc * HD, (cidx + 1) * hpc * HD)
        ks = slice(cidx * kpc * HD, (cidx + 1) * kpc * HD)
        wq_c = wq[qs].reshape(hpc, HD, DIM)[:, de, :].reshape(hpc * HD, DIM)
        wk_c = wk[ks].reshape(kpc, HD, DIM)[:, de, :].reshape(kpc * HD, DIM)
        wv_c = wv[ks]
        wkv_c = np.concatenate([wk_c, wv_c], axis=0)
        wo_c = wo[:, qs]
        in_maps.append({
            "x_t": x_t,
            "wq_t": np.ascontiguousarray(
                wq_c.T.reshape(NK, 128, hpc * HD).transpose(1, 0, 2)).astype(BF),
            "wkv_t": np.ascontiguousarray(
                wkv_c.T.reshape(NK, 128, KVDIM).transpose(1, 0, 2)).astype(BF),
            "wo_t": np.ascontiguousarray(wo_c.T).astype(BF),
            "cct": cct.astype(BF),
            "sst": sst.astype(BF),
            "ident": ident.astype(BF),
            "ones": ones.astype(BF),
            "mask": mask,
        })
    return in_maps


_CACHE = {}


def run(inputs, trace=False, trace_cores=None):
    x = np.asarray(inputs["x"], dtype=np.float32)
    B, S, _ = x.shape
    key = (B, S)
    if key not in _CACHE:
        _CACHE[key] = build_nc(B, S)
    nc = _CACHE[key]
    in_maps = host_prepare(
        x, np.asarray(inputs["wq"], np.float32), np.asarray(inputs["wk"], np.float32),
        np.asarray(inputs["wv"], np.float32), np.asarray(inputs["wo"], np.float32),
        np.asarray(inputs["freqs_cos"], np.float32),
        np.asarray(inputs["freqs_sin"], np.float32), B, S)
    res = bass_utils.run_bass_kernel_spmd(
        nc, in_maps, core_ids=list(range(N_CORES)), trace=trace,
        trace_cores=trace_cores)
    acc = np.zeros((B * S, DIM), dtype=np.float64)
    for r in res.results:
        acc += r["out"].astype(np.float64)
    out = acc.astype(np.float32).reshape(B, S, DIM)
    return out, res


def kernel(**inputs) -> np.ndarray:
    assert int(inputs.get("start_pos", 0)) == 0
    out, _ = run(inputs, trace=False)
    return out
